# revision 27
# baseline (speedup 1.0000x reference)
"""AssistedExcitation distributed Bass kernel for 8 TRN2 NeuronCores.

Reference computation (per batch b):
    mask[h,w]  = union over 32 boxes of axis-aligned rectangles (rasterized
                 from normalized xywh boxes, trunc + clamp semantics)
    att        = 5x5 conv of reflect-padded mask with the given kernel
    out        = x + att * x        (att broadcast over 256 channels)

Sharding: pure data parallel — batch 16 is split 2-per-core across 8 cores.
No collectives needed.

Per-core algorithm (all bulk work on-device):
  * Box preprocessing on the DVE reproduces the reference's exact f32
    arithmetic:  t1 = (c - wh*0.5)*80,  t2 = (c + wh*0.5)*80.
    For integer pixel p:  p >= max(0,trunc(t1)) <=> p > t1-1  and
    p <= min(79,trunc(t2)) <=> p <= t2, so interval indicators need no
    floor().  Validity (x2>x1 via trunc'd ints) == (#cols covered >= 2).
  * Rasterization is a matmul: indicator rows Cm[n,pw], Rv[n,ph] evaluated
    at reflect-mapped padded coordinates m[p]=min(|p-2|,158-(p-2)) give
    PT[pw,ph] = sum_n Cm*Rv via lhsT=Cm, rhs=Rv; binarize (>0) yields the
    *reflect-padded transposed* mask in one shot.
  * The 5x5 conv is 5 PSUM-accumulated matmuls with banded matrices
    Kc_i[pw,w] = k[i, pw-w]:  att[h,w] = sum_i sum_pw PT[pw,h+i]*Kc_i[pw,w].
    Kc (a pure repacking of the 25 kernel weights), the reflect-mapped
    coordinate row, and the broadcast ones-vector are precomputed host-side
    and shipped as small constant inputs — keeps the device critical path
    free of constant building.
  * (1+att) is broadcast across the 128 partitions with K=1 fp16 matmuls
    (lhsT = ones[1,128], rhs = fp16 flattened (1+att) row), evicted to
    SBUF f32, then out = x * att_bc on the VectorEngine, streamed in
    [128, 1600] chunks (double-buffered DMA in/out).

Scheduling notes (why the odd-looking dependency pins exist):
  * DMA completion waits honor the GLOBAL compile-time DMA order, so a
    small DMA scheduled after N megabyte x-chunks cannot be observed
    complete until all N have drained.  Hence: the single merged const
    DMA goes first on sync, and the per-batch att1->flat flatten DMAs
    are pinned between early x chunks via add_dep_helper.
  * The sync trigger stream is pinned to a prefetch-interleaved order
    (in0..in7, then [out_k, in_{k+8}]) so out-DMAs drain while the
    in-stream stays ahead, instead of the scheduler hoisting every
    in-trigger in front of the outs and starving the xout pool.
  * Both batches' boxes are processed in one 64-partition DVE pass to
    halve the serial attention-path latency before the first multiply.
"""

import numpy as np

import concourse.bass as bass
import concourse.tile as tile
from concourse import bacc, mybir
from concourse.tile_rust import add_dep_helper
from concourse.bass_utils import run_bass_kernel_spmd

F32 = mybir.dt.float32
F16 = mybir.dt.float16
ALU = mybir.AluOpType
ACT = mybir.ActivationFunctionType

N_CORES = 8
B, C, H, W, NBOX = 16, 256, 80, 80, 32
B_LOC = B // N_CORES          # 2 batches per core
HW = H * W                    # 6400
PAD = 84                      # 80 + 2*2 reflect pad
KS = 5
CH = 1600                     # free-dim chunk of the x stream
N_CHUNK = HW // CH            # 4
BC_CH = 512                   # psum bank width for the broadcast matmul


def _build_nc():
    nc = bacc.Bacc(None, target_bir_lowering=False)

    x_d = nc.declare_dram_parameter("x", [B_LOC, C, H, W], F32, isOutput=False)
    boxes_d = nc.declare_dram_parameter("boxes", [B_LOC, NBOX, 4], F32, isOutput=False)
    nc.declare_dram_parameter("kernel", [1, 1, KS, KS], F32, isOutput=False)
    # single merged const tensor, f16-typed; the f32 piece is bitcast back
    CST_COLS = (KS * W + 128) + 2 * (PAD + 4)
    cst_d = nc.declare_dram_parameter("cst", [PAD, CST_COLS], F16, isOutput=False)
    out_d = nc.declare_dram_parameter("out", [B_LOC, C, H, W], F32, isOutput=True)

    xr = x_d.rearrange("b c h w -> b c (h w)")
    outr = out_d.rearrange("b c h w -> b c (h w)")

    with tile.TileContext(nc) as tc:
        with (
            tc.tile_pool(name="const", bufs=1) as cp,
            tc.tile_pool(name="batch", bufs=2) as bp,
            tc.tile_pool(name="attbc", bufs=2) as ap_,
            tc.tile_pool(name="xin", bufs=10) as xp,
            tc.tile_pool(name="xout", bufs=8) as op_,
            tc.tile_pool(name="ps_small", bufs=2, space=bass.MemorySpace.PSUM) as psm,
            tc.tile_pool(name="ps_bc", bufs=4, space=bass.MemorySpace.PSUM) as pbc,
        ):
            # One merged const DMA, triggered on sync BEFORE the x flood:
            # DMA completion waits honor global trigger order, so it must
            # precede the megabyte x chunks. Contents: banded conv matrices,
            # ones row, and (f32-bitcast) mapped coords + both batches' boxes.
            NB2 = B_LOC * NBOX
            cst = cp.tile([PAD, CST_COLS], F16)
            nc.sync.dma_start(cst[:], cst_d[:])
            kc = cst[:, 0 : KS * W]
            ones16 = cst[0:1, KS * W : KS * W + 128]
            c32 = cst[0:NB2, KS * W + 128 : CST_COLS].bitcast(F32)  # [64, 88] f32
            mapped = c32[:, 0:PAD]
            bx = c32[:, PAD : PAD + 4]

            # ---- box preprocessing for BOTH batches in one 64-partition pass
            half = cp.tile([NB2, 2], F32)
            nc.vector.tensor_scalar(half[:], bx[:, 2:4], 0.5, None, op0=ALU.mult)
            t1 = cp.tile([NB2, 2], F32)
            nc.vector.tensor_tensor(t1[:], bx[:, 0:2], half[:], op=ALU.subtract)
            nc.vector.tensor_scalar(t1[:], t1[:], float(W), None, op0=ALU.mult)
            t2 = cp.tile([NB2, 2], F32)
            nc.vector.tensor_tensor(t2[:], bx[:, 0:2], half[:], op=ALU.add)
            nc.vector.tensor_scalar(t2[:], t2[:], float(W), None, op0=ALU.mult)
            t1m = cp.tile([NB2, 2], F32)
            nc.vector.tensor_scalar(t1m[:], t1[:], -1.0, None, op0=ALU.add)

            cm = cp.tile([NB2, PAD], F16)
            nc.vector.tensor_scalar(cm[:], mapped[:], t1m[:, 0:1], None, op0=ALU.is_gt)
            nc.vector.scalar_tensor_tensor(
                cm[:], mapped[:], t2[:, 0:1], cm[:], op0=ALU.is_le, op1=ALU.mult
            )
            rm = cp.tile([NB2, PAD], F16)
            nc.vector.tensor_scalar(rm[:], mapped[:], t1m[:, 1:2], None, op0=ALU.is_gt)
            nc.vector.scalar_tensor_tensor(
                rm[:], mapped[:], t2[:, 1:2], rm[:], op0=ALU.is_le, op1=ALU.mult
            )

            rowc = cp.tile([NB2, 1], F32)
            nc.vector.tensor_reduce(rowc[:], rm[:, 2:82], axis=mybir.AxisListType.X, op=ALU.add)
            colc = cp.tile([NB2, 1], F32)
            nc.vector.tensor_reduce(colc[:], cm[:, 2:82], axis=mybir.AxisListType.X, op=ALU.add)
            vv = cp.tile([NB2, 1], F32)
            nc.vector.tensor_scalar(vv[:], rowc[:], 1.5, None, op0=ALU.is_ge)
            nc.vector.scalar_tensor_tensor(
                vv[:], colc[:], 1.5, vv[:], op0=ALU.is_ge, op1=ALU.mult
            )
            rv = cp.tile([NB2, PAD], F16)
            nc.vector.tensor_scalar(rv[:], rm[:], vv[:], None, op0=ALU.mult)

            # ---------------- per-batch attention pipeline ----------------
            att_bcs = []
            att1_acts = []
            flat_trigs = []
            for b in range(B_LOC):
                # rasterize: PT[pw, ph] = #boxes covering the (padded) pixel
                pt_ps = psm.tile([PAD, PAD], F32, tag="pt_ps")
                nc.tensor.matmul(
                    pt_ps[:],
                    cm[b * NBOX : (b + 1) * NBOX, :],
                    rv[b * NBOX : (b + 1) * NBOX, :],
                    start=True, stop=True,
                )
                ptm = bp.tile([PAD, PAD], F16)
                nc.vector.tensor_scalar(ptm[:], pt_ps[:], 0.5, None, op0=ALU.is_ge)

                # 5x5 conv: 5 accumulated matmuls
                att_ps = psm.tile([H, W], F32, tag="att_ps")
                for i in range(KS):
                    nc.tensor.matmul(
                        att_ps[:],
                        ptm[:, i : i + H],
                        kc[:, i * W : (i + 1) * W],
                        start=(i == 0),
                        stop=(i == KS - 1),
                    )
                # (1 + att), cast to fp16 for the cheap broadcast matmul
                att1 = bp.tile([H, W], F16)
                att1_act = nc.scalar.activation(att1[:], att_ps[:], ACT.Copy, bias=1.0)
                att1_acts.append(att1_act)

                # flatten [80,80] -> [1,6400], broadcast across partitions via
                # K=1 fp16 matmuls, evict psum -> SBUF f32
                flat = bp.tile([1, HW], F16)
                flat_trig = nc.scalar.dma_start(flat[:], att1[:])
                flat_trigs.append(flat_trig)
                att_bc = ap_.tile([128, HW], F32, tag="att_bc")
                off = 0
                ci = 0
                while off < HW:
                    cw = min(BC_CH, HW - off)
                    bc_ps = pbc.tile([128, BC_CH], F32, tag="bc_ps")
                    nc.tensor.matmul(
                        bc_ps[:, 0:cw], ones16[:], flat[:, off : off + cw],
                        start=True, stop=True,
                    )
                    if b == 0 and ci % 2 == 1:
                        nc.vector.tensor_copy(att_bc[:, off : off + cw], bc_ps[:, 0:cw])
                    else:
                        nc.scalar.copy(att_bc[:, off : off + cw], bc_ps[:, 0:cw])
                    off += cw
                    ci += 1
                att_bcs.append(att_bc)

            # ---------------- main stream: out = x * (1 + att) ----------------
            # Emit triggers in prefetch-interleaved order and PIN that order
            # on the sync engine (ordering-only deps): in0..in{P-1}, then
            # [mult_k, out_k, in_{k+P}] — keeps out-DMAs draining while the
            # in-stream stays P chunks ahead, without the scheduler hoisting
            # every in-trigger in front of the outs.
            chunks = [
                (b, chalf * 128, k * CH, CH)
                for b in range(B_LOC)
                for chalf in range(C // 128)
                for k in range(N_CHUNK)
            ]
            # split the final chunk in two so the tail (last-in -> mult ->
            # out) pipelines at half granularity instead of serializing 4us
            lb, lc0, lo0, _ = chunks[-1]
            chunks = chunks[:-1] + [(lb, lc0, lo0, CH // 2),
                                    (lb, lc0, lo0 + CH // 2, CH // 2)]
            PREF = 8
            trig_chain = []

            def _chain(bi):
                if trig_chain:
                    add_dep_helper(bi.ins, trig_chain[-1].ins, sync=False,
                                   reason="pin sync trigger order")
                trig_chain.append(bi)

            xts = {}
            in_trigs = []

            def _load(i):
                b, c0, o0, w = chunks[i]
                xt = xp.tile([128, w], F32, name=f"xt{i}", tag="xt")
                bi = nc.sync.dma_start(xt[:], xr[b, c0 : c0 + 128, o0 : o0 + w])
                _chain(bi)
                in_trigs.append(bi)
                xts[i] = xt

            for i in range(PREF):
                _load(i)

            # Keep the flatten DMAs EARLY in the global (compile-time) DMA
            # order: their completion waits count every previously scheduled
            # DMA, so behind the x flood they'd complete ~10us late.
            # flat0 goes between in1 and in2, flat1 between in3 and in4.
            add_dep_helper(in_trigs[3].ins, flat_trigs[0].ins, sync=True,
                           reason="flat0 ahead of x flood")
            add_dep_helper(in_trigs[5].ins, flat_trigs[1].ins, sync=True,
                           reason="flat1 ahead of x flood")
            # flat0's trigger precedes b1's att1 on the scalar stream
            add_dep_helper(att1_acts[1].ins, flat_trigs[0].ins, sync=False,
                           reason="flat0 before att1_1 on ACT")
            for i, (b, c0, o0, w) in enumerate(chunks):
                xt = xts.pop(i)
                ot = op_.tile([128, w], F32, name=f"ot{i}", tag="ot")
                nc.vector.tensor_mul(ot[:], xt[:], att_bcs[b][:, o0 : o0 + w])
                _chain(nc.sync.dma_start(outr[b, c0 : c0 + 128, o0 : o0 + w], ot[:]))
                if i + PREF < len(chunks):
                    _load(i + PREF)

    if not nc.is_finalized():
        nc.finalize()
    return nc


def _host_consts(ker: np.ndarray, boxes_shard: np.ndarray):
    """Host-side repacking of the 5x5 kernel + compile-time constants.
    cst32 [32, 84+8]: reflect-mapped padded coords | per-batch boxes.
    cst16 [84, 400+128]: banded conv matrices Kc | ones row (partition 0)."""
    k = ker.reshape(KS, KS).astype(np.float32)
    cst16 = np.zeros((PAD, KS * W + 128), dtype=np.float16)
    for i in range(KS):
        for j in range(KS):
            w = np.arange(W)
            cst16[w + j, i * W + w] = np.float16(k[i, j])
    cst16[0, KS * W : KS * W + 128] = np.float16(1.0)
    p = np.arange(PAD, dtype=np.float32)
    mapped_row = np.minimum(np.abs(p - 2.0), 158.0 - (p - 2.0)).astype(np.float32)
    cst32 = np.zeros((B_LOC * NBOX, PAD + 4), dtype=np.float32)
    cst32[:, 0:PAD] = mapped_row[None, :]
    cst32[:, PAD : PAD + 4] = boxes_shard.reshape(B_LOC * NBOX, 4)
    cst = np.zeros((PAD, (KS * W + 128) + 2 * (PAD + 4)), dtype=np.float16)
    cst[:, 0 : KS * W + 128] = cst16
    cst[0 : B_LOC * NBOX, KS * W + 128 :] = cst32.view(np.float16)
    return cst


_NC_CACHE = None


def _get_nc():
    global _NC_CACHE
    if _NC_CACHE is None:
        _NC_CACHE = _build_nc()
    return _NC_CACHE


def _run(inputs, trace=False, **kw):
    x = np.ascontiguousarray(np.asarray(inputs["x"], dtype=np.float32))
    boxes = np.ascontiguousarray(np.asarray(inputs["boxes"], dtype=np.float32))
    ker = np.ascontiguousarray(np.asarray(inputs["kernel"], dtype=np.float32))
    assert x.shape == (B, C, H, W) and boxes.shape == (B, NBOX, 4)

    nc = _get_nc()
    in_maps = []
    for i in range(N_CORES):
        bsh = boxes[i * B_LOC : (i + 1) * B_LOC]
        cst = _host_consts(ker, bsh)
        in_maps.append(
            {
                "x": x[i * B_LOC : (i + 1) * B_LOC],
                "boxes": bsh,
                "kernel": ker,
                "cst": cst,
            }
        )
    res = run_bass_kernel_spmd(nc, in_maps, core_ids=list(range(N_CORES)),
                               trace=trace, **kw)
    out = np.concatenate([r["out"] for r in res.results], axis=0)
    return out, res


def kernel(**inputs) -> np.ndarray:
    out, _ = _run(inputs, trace=False)
    return out
